# revision 15
# baseline (speedup 1.0000x reference)
"""Trainium2 Bass kernel for nn_SinePredictor (gnn_message_passing).

reference:
    s = h[edges[0]]; o = h[edges[1]]
    score = sin(s - o) @ W.T + b          # [E, 1]
    score = softmax(score.reshape(-1,2), axis=1).reshape(-1,1)
    return (score, score > 0.5)

Strategy (8 NeuronCores, SPMD):
  - Shard pairs (consecutive edge couples) across 8 cores; replicate h, W.
  - Device-side row gather via gpsimd.dma_gather (int16 indices).  h has
    100000 rows > int16 range, so two 65536-row windows with mid-table
    biased bases (signed int16 reach) cover it:
        W0: base row 32768 -> rows [0, 65536)
        W1: base row 67232 -> rows [34464, 100000)
  - dma_gather requires each instruction's index list to be window-pure, so
    pairs are bucket-sorted host-side by the 4-bit key (window of even-s,
    even-o, odd-s, odd-o).  Even edges live in slot-space A, odd edges in
    slot-space B, aligned by pair slot, so the pairwise softmax is a pure
    elementwise op between the two spaces.  Final outputs are unpermuted on
    the host (pure data movement).
  - Four gather streams (A-s, A-o, B-s, B-o) ride four SWDGE queues
    (descriptor generation runs on different Q7 core pairs in parallel).
  - sin: ACT table is ~1 ulp for |x| <= pi only; args reach +-8.4, so
    range-reduce with k = round(x/(2pi)) via the +-1.5*2^23 magic-add trick,
    r = x - k*f32(2pi).
  - only the pair difference d2 is ever needed: (sin(dA)-sin(dB)) * W,
    reduced over D on DVE (one mul+reduce instead of two score pipelines).
  - pair softmax: softmax([a,b]) = [sigmoid(a-b), sigmoid(b-a)]; bool output
    from the sign of d2 = a-b directly (exactly matches p > 0.5).
"""
import numpy as np

import concourse.bacc as bacc
import concourse.mybir as mybir
import concourse.tile as tile
from concourse.bass_utils import run_bass_kernel_spmd

NCORES = 8
P = 128
D = 128
N_NODES = 100000
B0 = 32768            # window-0 base row (covers rows [0, 65536))
B1 = 67232            # window-1 base row (covers rows [34464, 100000))
WIN_SPLIT = 65536     # rows < WIN_SPLIT use window 0, else window 1
WSLOTS = 896          # pair slots per compute window; len+1 sentinel keeps each
                      # dma_gather at <= 8 x 128-blocks (65 descs/engine, HW-safe)
BLK = WSLOTS // P     # 128-slot blocks per window (8)

MAGIC = 12582912.0            # 1.5 * 2^23: add+sub rounds f32 to nearest int
INV2PI = float(np.float32(1.0 / (2.0 * np.pi)))
C1 = 6.28125                  # exact in f32 (11-bit mantissa)
C2 = float(np.float32(2.0 * np.pi - 6.28125))
C1F = float(np.float32(2.0 * np.pi))   # single-step Cody-Waite constant

F32 = mybir.dt.float32
I16 = mybir.dt.int16
I8 = mybir.dt.int8
AF = mybir.ActivationFunctionType
ALU = mybir.AluOpType


def _split_sync_waits(nc, max_waits=1):
    """Walrus limits sync-wait slots per instruction (1 for DMA structs).
    Move excess waits onto preceding same-engine NoOps."""
    for func in nc.m.functions:
        for blk in func.blocks:
            insts = blk.instructions
            i = 0
            while i < len(insts):
                inst = insts[i]
                si = inst.sync_info
                if si is not None and len(si.on_wait) > max_waits:
                    waits = list(si.on_wait)
                    keep = waits[-max_waits:]
                    excess = waits[:-max_waits]
                    n_nops = 0
                    for cs in range(0, len(excess), max_waits):
                        chunk = excess[cs:cs + max_waits]
                        nop = mybir.InstNoOp(
                            name=nc.get_next_instruction_name(), ins=[], outs=[])
                        nop.engine = inst.engine
                        nop.sync_info = mybir.SyncInfo(on_wait=chunk, on_update=[])
                        nc.register_instruction(nop)
                        insts.insert(i + n_nops, nop)
                        n_nops += 1
                    si.on_wait = keep
                    i += n_nops
                i += 1
    return nc


def _build_program(s_total, insts, idx_cols, n_nodes=N_NODES, bases=(B0, B1), bias_val=0.0):
    """Build the SPMD Bass program.

    insts: list of dicts(w, s, a, len, bit, col) — identical across cores.
    """
    n_win = s_total // WSLOTS
    sc_cols = s_total // P

    nc = bacc.Bacc(None, target_bir_lowering=False, num_swdge_queues=4)

    h = nc.dram_tensor("h", [n_nodes, D], F32, kind="ExternalInput")
    idx = nc.dram_tensor("idx", [P, idx_cols], I16, kind="ExternalInput")
    wrep = nc.dram_tensor("wrep", [P, WSLOTS], F32, kind="ExternalInput")

    pe_out = nc.dram_tensor("pe_out", [P, sc_cols], F32, kind="ExternalOutput")
    po_out = nc.dram_tensor("po_out", [P, sc_cols], F32, kind="ExternalOutput")
    me_out = nc.dram_tensor("me_out", [P, sc_cols], I8, kind="ExternalOutput")
    mo_out = nc.dram_tensor("mo_out", [P, sc_cols], I8, kind="ExternalOutput")

    h_base = [h[bases[0]:, :], h[bases[1]:, :]]

    by_win = [[] for _ in range(n_win)]
    for it in insts:
        by_win[it["w"]].append(it)

    with tile.TileContext(nc) as tc:
        with tc.tile_pool(name="cst", bufs=1) as cst, \
             tc.tile_pool(name="gpool", bufs=6) as gpool, \
             tc.tile_pool(name="pool", bufs=4) as pool:
            idx_t = cst.tile([P, idx_cols], I16, name="idx_t")
            nc.sync.dma_start(out=idx_t[:, :], in_=idx[:, :])
            wrep_t = cst.tile([P, WSLOTS], F32, name="wrep_t")
            nc.sync.dma_start(out=wrep_t[:, :], in_=wrep[:, :])

            d2 = cst.tile([P, sc_cols], F32, name="d2")

            for w in range(n_win):
                # gather tiles: one spare block for the sentinel slot
                g = {}
                for sname, snum in (("As", 0), ("Ao", 1), ("Bs", 2), ("Bo", 3)):
                    g[snum] = gpool.tile([P, BLK + 1, D], F32,
                                         name=f"g{sname}", tag=f"g{sname}")
                for it in by_win[w]:
                    nblk = it["len"] // P
                    nc.gpsimd.dma_gather(
                        out_ap=g[it["s"]][:, it["a"]:it["a"] + nblk + 1, :],
                        in_ap=h_base[it["bit"]],
                        idxs_ap=idx_t[:, it["col"]:it["col"] + it["len"] // 16 + 1],
                        num_idxs=it["len"] + 1,
                        num_idxs_reg=it["len"] + 1,
                        elem_size=D,
                        queue_num=it["s"],
                    )

                sins = []
                for sp, (gs, go) in enumerate(((0, 1), (2, 3))):
                    dt = pool.tile([P, WSLOTS], F32, name=f"d{sp}", tag=f"d{sp}")
                    tt = pool.tile([P, WSLOTS], F32, name=f"t{sp}", tag=f"t{sp}")
                    gsv = g[gs].rearrange("p a b -> p (a b)")
                    gov = g[go].rearrange("p a b -> p (a b)")
                    # d = s - o
                    nc.vector.tensor_sub(dt[:, :], gsv[:, :WSLOTS], gov[:, :WSLOTS])
                    # t = d*inv2pi + MAGIC ; k = t - MAGIC (both on ACT)
                    nc.scalar.activation(tt[:, :], dt[:, :], AF.Copy,
                                         bias=MAGIC, scale=INV2PI)
                    nc.scalar.activation(tt[:, :], tt[:, :], AF.Copy,
                                         bias=-MAGIC, scale=1.0)
                    # r = d - k*(C1+C2)  (single-step Cody-Waite, in place)
                    nc.vector.scalar_tensor_tensor(
                        out=dt[:, :], in0=tt[:, :], scalar=-C1F, in1=dt[:, :],
                        op0=ALU.mult, op1=ALU.add)
                    # sin
                    nc.scalar.activation(tt[:, :], dt[:, :], AF.Sin)
                    sins.append((dt, tt))
                # d2 terms: (sinA - sinB) * W, reduced over D
                dA, sA = sins[0]
                dB, sB = sins[1]
                nc.vector.tensor_sub(dA[:, :], sA[:, :], sB[:, :])
                nc.vector.tensor_mul(dA[:, :], dA[:, :], wrep_t[:, :])
                nc.vector.tensor_reduce(
                    out=d2[:, w * BLK:(w + 1) * BLK],
                    in_=dA.rearrange("p (a b) -> p a b", b=D),
                    axis=mybir.AxisListType.X, op=ALU.add)

            # phase 2: pairwise softmax straight from d2 (bias cancels)
            pe = cst.tile([P, sc_cols], F32, name="pe")
            po = cst.tile([P, sc_cols], F32, name="po")
            nc.scalar.activation(pe[:, :], d2[:, :], AF.Sigmoid)
            nc.scalar.activation(po[:, :], d2[:, :], AF.Sigmoid, scale=-1.0)
            me = cst.tile([P, sc_cols], I8, name="me")
            mo = cst.tile([P, sc_cols], I8, name="mo")
            nc.vector.tensor_scalar(me[:, :], d2[:, :], 0.0, None, ALU.is_gt)
            nc.vector.tensor_scalar(mo[:, :], d2[:, :], 0.0, None, ALU.is_lt)

            nc.sync.dma_start(out=pe_out[:, :], in_=pe[:, :])
            nc.sync.dma_start(out=po_out[:, :], in_=po[:, :])
            nc.sync.dma_start(out=me_out[:, :], in_=me[:, :])
            nc.sync.dma_start(out=mo_out[:, :], in_=mo[:, :])

    nc.compile()
    return _split_sync_waits(nc)


def _plan(rows_asaobsbo_by_core):
    """Shared (SPMD) instruction layout from per-core bucket counts.

    rows_asaobsbo_by_core: per core dict with 'key' [npairs] int arrays.
    Returns caps, bucket offsets, s_total, inst list, idx_cols.
    """
    counts = np.zeros((NCORES, 16), np.int64)
    for c in range(NCORES):
        counts[c] = np.bincount(rows_asaobsbo_by_core[c]["key"], minlength=16)
    caps = ((counts.max(axis=0) + P - 1) // P * P).astype(np.int64)
    caps = np.maximum(caps, P)
    s_used = int(caps.sum())
    s_total = (s_used + WSLOTS - 1) // WSLOTS * WSLOTS
    caps[15] += s_total - s_used
    offs = np.zeros(17, np.int64)
    offs[1:] = np.cumsum(caps)

    # stream s: key bit (3-s); window bit of bucket b for stream s:
    def bit(b, s):
        return (b >> (3 - s)) & 1

    insts = []
    idx_cols = 0
    n_win = s_total // WSLOTS
    for w in range(n_win):
        lo_w, hi_w = w * WSLOTS, (w + 1) * WSLOTS
        for s in range(4):
            # runs of consecutive buckets with equal bit
            b = 0
            while b < 16:
                e = b
                while e + 1 < 16 and bit(e + 1, s) == bit(b, s):
                    e += 1
                rlo, rhi = int(offs[b]), int(offs[e + 1])
                a, bnd = max(rlo, lo_w), min(rhi, hi_w)
                if a < bnd:
                    ln = bnd - a
                    insts.append(dict(w=w, s=s, a=(a - lo_w) // P, len=ln,
                                      bit=bit(b, s), col=idx_cols))
                    idx_cols += ln // 16 + 1
                b = e + 1
    return caps, offs, s_total, insts, idx_cols


def kernel(h, edges, W, b):
    h = np.ascontiguousarray(np.asarray(h, dtype=np.float32))
    edges = np.asarray(edges)
    W = np.asarray(W, dtype=np.float32)
    b = np.asarray(b, dtype=np.float32)
    E = edges.shape[1]
    npairs = E // 2
    pp_core = npairs // NCORES
    assert npairs % NCORES == 0

    ev_s = edges[0, 0::2].astype(np.int64)
    ev_o = edges[1, 0::2].astype(np.int64)
    od_s = edges[0, 1::2].astype(np.int64)
    od_o = edges[1, 1::2].astype(np.int64)

    win = lambda r: (r >= WIN_SPLIT).astype(np.int64)
    key_all = (win(ev_s) << 3) | (win(ev_o) << 2) | (win(od_s) << 1) | win(od_o)

    per_core = []
    for c in range(NCORES):
        sl = slice(c * pp_core, (c + 1) * pp_core)
        per_core.append({
            "key": key_all[sl],
            "rows": (ev_s[sl], ev_o[sl], od_s[sl], od_o[sl]),
            "orig": np.arange(c * pp_core, (c + 1) * pp_core, dtype=np.int64),
        })

    caps, offs, s_total, insts, idx_cols = _plan(per_core)
    n_win = s_total // WSLOTS
    sc_cols = s_total // P

    bases = np.array([B0, B1], np.int64)

    # per-core slot assignment + idx blobs
    in_maps = []
    slot_orig = []  # per core: orig pair id per slot (-1 = pad)
    wrep_np = np.tile(W[0], (P, 1)).astype(np.float32)
    wrep_np = np.tile(wrep_np, (1, WSLOTS // D))  # [P, WSLOTS]

    for c in range(NCORES):
        pc = per_core[c]
        order = np.argsort(pc["key"], kind="stable")
        keys_sorted = pc["key"][order]
        # slot for i-th sorted pair: bucket offset + rank within bucket
        kcounts = np.bincount(keys_sorted, minlength=16)
        koffs = np.zeros(16, np.int64)
        koffs[:] = offs[:16]
        rank = np.arange(len(order)) - np.repeat(
            np.cumsum(np.concatenate([[0], kcounts[:-1]])), kcounts)
        slots = koffs[keys_sorted] + rank

        so = np.full(s_total, -1, np.int64)
        so[slots] = pc["orig"][order]
        slot_orig.append(so)

        # per-stream row per slot (pads: row = base row of the bucket's window)
        rows_slot = np.zeros((4, s_total), np.int64)
        for s in range(4):
            rows_slot[s, slots] = pc["rows"][s][order]
        # pads: fill with a row valid for each bucket's window for that stream
        pad_mask = so < 0
        if pad_mask.any():
            bucket_of_slot = np.searchsorted(offs[1:17], np.arange(s_total),
                                             side="right")
            for s in range(4):
                bit_s = (bucket_of_slot >> (3 - s)) & 1
                rows_slot[s, pad_mask] = bases[bit_s[pad_mask]]

        blob = np.zeros((P, idx_cols), np.int16)
        for it in insts:
            s = it["s"]
            lo = it["w"] * WSLOTS + it["a"] * P
            loc = rows_slot[s, lo:lo + it["len"]] - bases[it["bit"]]
            block = np.zeros((16, it["len"] // 16 + 1), np.int16)
            block[:, :-1] = loc.astype(np.int16).reshape(-1, 16).T
            block[0, -1] = 0  # sentinel: keeps trailing index non-negative
            blob[:, it["col"]:it["col"] + block.shape[1]] = np.tile(block, (8, 1))

        in_maps.append({"h": h, "idx": blob, "wrep": wrep_np})

    bias_val = float(b.reshape(-1)[0]) if b.size else 0.0
    nc = _build_program(s_total, insts, idx_cols, n_nodes=h.shape[0],
                        bases=(B0, B1), bias_val=bias_val)
    res = run_bass_kernel_spmd(nc, in_maps, core_ids=list(range(NCORES)))

    score = np.zeros(E, np.float32)
    mask = np.zeros(E, np.uint8)
    for c in range(NCORES):
        out = res.results[c]
        # [P, sc_cols] -> slot order: slot = (col//BLK)*WSLOTS + (col%BLK)*P + p
        def to_slots(arr):
            return arr.reshape(P, n_win, BLK).transpose(1, 2, 0).reshape(-1)
        pe = to_slots(out["pe_out"])
        po = to_slots(out["po_out"])
        me = to_slots(out["me_out"])
        mo = to_slots(out["mo_out"])
        so = slot_orig[c]
        v = so >= 0
        op = so[v]
        score[2 * op] = pe[v]
        score[2 * op + 1] = po[v]
        mask[2 * op] = me[v]
        mask[2 * op + 1] = mo[v]

    return score.reshape(E, 1), mask.reshape(E, 1).astype(bool)


# revision 16
# speedup vs baseline: 1.0010x; 1.0010x over previous
"""Trainium2 Bass kernel for nn_SinePredictor (gnn_message_passing).

reference:
    s = h[edges[0]]; o = h[edges[1]]
    score = sin(s - o) @ W.T + b          # [E, 1]
    score = softmax(score.reshape(-1,2), axis=1).reshape(-1,1)
    return (score, score > 0.5)

Strategy (8 NeuronCores, SPMD):
  - Shard pairs (consecutive edge couples) across 8 cores; replicate h, W.
  - Device-side row gather via gpsimd.dma_gather (int16 indices).  h has
    100000 rows > int16 range, so two 65536-row windows with mid-table
    biased bases (signed int16 reach) cover it:
        W0: base row 32768 -> rows [0, 65536)
        W1: base row 67232 -> rows [34464, 100000)
  - dma_gather requires each instruction's index list to be window-pure, so
    pairs are bucket-sorted host-side by the 4-bit key (window of even-s,
    even-o, odd-s, odd-o).  Even edges live in slot-space A, odd edges in
    slot-space B, aligned by pair slot, so the pairwise softmax is a pure
    elementwise op between the two spaces.  Final outputs are unpermuted on
    the host (pure data movement).
  - Four gather streams (A-s, A-o, B-s, B-o) ride four SWDGE queues
    (descriptor generation runs on different Q7 core pairs in parallel).
  - sin: ACT table is ~1 ulp for |x| <= pi only; args reach +-8.4, so
    range-reduce with k = round(x/(2pi)) via the +-1.5*2^23 magic-add trick,
    r = x - k*f32(2pi).
  - only the pair difference d2 is ever needed: (sin(dA)-sin(dB)) * W,
    reduced over D on DVE (one mul+reduce instead of two score pipelines).
  - pair softmax: softmax([a,b]) = [sigmoid(a-b), sigmoid(b-a)]; bool output
    from the sign of d2 = a-b directly (exactly matches p > 0.5).
"""
import numpy as np

import concourse.bacc as bacc
import concourse.mybir as mybir
import concourse.tile as tile
from concourse.bass_utils import run_bass_kernel_spmd

NCORES = 8
P = 128
D = 128
N_NODES = 100000
B0 = 32768            # window-0 base row (covers rows [0, 65536))
B1 = 67232            # window-1 base row (covers rows [34464, 100000))
WIN_SPLIT = 65536     # rows < WIN_SPLIT use window 0, else window 1
WSLOTS = 896          # pair slots per compute window; len+1 sentinel keeps each
                      # dma_gather at <= 8 x 128-blocks (65 descs/engine, HW-safe)
BLK = WSLOTS // P     # 128-slot blocks per window (8)

MAGIC = 12582912.0            # 1.5 * 2^23: add+sub rounds f32 to nearest int
INV2PI = float(np.float32(1.0 / (2.0 * np.pi)))
C1 = 6.28125                  # exact in f32 (11-bit mantissa)
C2 = float(np.float32(2.0 * np.pi - 6.28125))
C1F = float(np.float32(2.0 * np.pi))   # single-step Cody-Waite constant

F32 = mybir.dt.float32
I16 = mybir.dt.int16
I8 = mybir.dt.int8
AF = mybir.ActivationFunctionType
ALU = mybir.AluOpType


def _split_sync_waits(nc, max_waits=1):
    """Walrus limits sync-wait slots per instruction (1 for DMA structs).
    Move excess waits onto preceding same-engine NoOps."""
    for func in nc.m.functions:
        for blk in func.blocks:
            insts = blk.instructions
            i = 0
            while i < len(insts):
                inst = insts[i]
                si = inst.sync_info
                if si is not None and len(si.on_wait) > max_waits:
                    waits = list(si.on_wait)
                    keep = waits[-max_waits:]
                    excess = waits[:-max_waits]
                    n_nops = 0
                    for cs in range(0, len(excess), max_waits):
                        chunk = excess[cs:cs + max_waits]
                        nop = mybir.InstNoOp(
                            name=nc.get_next_instruction_name(), ins=[], outs=[])
                        nop.engine = inst.engine
                        nop.sync_info = mybir.SyncInfo(on_wait=chunk, on_update=[])
                        nc.register_instruction(nop)
                        insts.insert(i + n_nops, nop)
                        n_nops += 1
                    si.on_wait = keep
                    i += n_nops
                i += 1
    return nc


def _build_program(s_total, insts, idx_cols, n_nodes=N_NODES, bases=(B0, B1), bias_val=0.0):
    """Build the SPMD Bass program.

    insts: list of dicts(w, s, a, len, bit, col) — identical across cores.
    """
    n_win = s_total // WSLOTS
    sc_cols = s_total // P

    nc = bacc.Bacc(None, target_bir_lowering=False, num_swdge_queues=4)

    h = nc.dram_tensor("h", [n_nodes, D], F32, kind="ExternalInput")
    idx = nc.dram_tensor("idx", [P, idx_cols], I16, kind="ExternalInput")
    wrep = nc.dram_tensor("wrep", [P, WSLOTS], F32, kind="ExternalInput")

    pe_out = nc.dram_tensor("pe_out", [P, sc_cols], F32, kind="ExternalOutput")
    po_out = nc.dram_tensor("po_out", [P, sc_cols], F32, kind="ExternalOutput")
    me_out = nc.dram_tensor("me_out", [P, sc_cols], I8, kind="ExternalOutput")
    mo_out = nc.dram_tensor("mo_out", [P, sc_cols], I8, kind="ExternalOutput")

    h_base = [h[bases[0]:, :], h[bases[1]:, :]]

    by_win = [[] for _ in range(n_win)]
    for it in insts:
        by_win[it["w"]].append(it)

    with tile.TileContext(nc) as tc:
        with tc.tile_pool(name="cst", bufs=1) as cst, \
             tc.tile_pool(name="gpool", bufs=7) as gpool, \
             tc.tile_pool(name="pool", bufs=3) as pool:
            idx_t = cst.tile([P, idx_cols], I16, name="idx_t")
            nc.sync.dma_start(out=idx_t[:, :], in_=idx[:, :])
            wrep_t = cst.tile([P, WSLOTS], F32, name="wrep_t")
            nc.sync.dma_start(out=wrep_t[:, :], in_=wrep[:, :])

            d2 = cst.tile([P, sc_cols], F32, name="d2")

            for w in range(n_win):
                # gather tiles: one spare block for the sentinel slot
                g = {}
                for sname, snum in (("As", 0), ("Ao", 1), ("Bs", 2), ("Bo", 3)):
                    g[snum] = gpool.tile([P, BLK + 1, D], F32,
                                         name=f"g{sname}", tag=f"g{sname}")
                for it in by_win[w]:
                    nblk = it["len"] // P
                    nc.gpsimd.dma_gather(
                        out_ap=g[it["s"]][:, it["a"]:it["a"] + nblk + 1, :],
                        in_ap=h_base[it["bit"]],
                        idxs_ap=idx_t[:, it["col"]:it["col"] + it["len"] // 16 + 1],
                        num_idxs=it["len"] + 1,
                        num_idxs_reg=it["len"] + 1,
                        elem_size=D,
                        queue_num=it["s"],
                    )

                sins = []
                for sp, (gs, go) in enumerate(((0, 1), (2, 3))):
                    dt = pool.tile([P, WSLOTS], F32, name=f"d{sp}", tag=f"d{sp}")
                    tt = pool.tile([P, WSLOTS], F32, name=f"t{sp}", tag=f"t{sp}")
                    gsv = g[gs].rearrange("p a b -> p (a b)")
                    gov = g[go].rearrange("p a b -> p (a b)")
                    # d = s - o
                    nc.vector.tensor_sub(dt[:, :], gsv[:, :WSLOTS], gov[:, :WSLOTS])
                    # t = d*inv2pi + MAGIC ; k = t - MAGIC (both on ACT)
                    nc.scalar.activation(tt[:, :], dt[:, :], AF.Copy,
                                         bias=MAGIC, scale=INV2PI)
                    nc.scalar.activation(tt[:, :], tt[:, :], AF.Copy,
                                         bias=-MAGIC, scale=1.0)
                    # r = d - k*(C1+C2)  (single-step Cody-Waite, in place)
                    nc.vector.scalar_tensor_tensor(
                        out=dt[:, :], in0=tt[:, :], scalar=-C1F, in1=dt[:, :],
                        op0=ALU.mult, op1=ALU.add)
                    # sin
                    nc.scalar.activation(tt[:, :], dt[:, :], AF.Sin)
                    sins.append((dt, tt))
                # d2 terms: (sinA - sinB) * W, reduced over D
                dA, sA = sins[0]
                dB, sB = sins[1]
                nc.vector.tensor_sub(dA[:, :], sA[:, :], sB[:, :])
                nc.vector.tensor_mul(dA[:, :], dA[:, :], wrep_t[:, :])
                nc.vector.tensor_reduce(
                    out=d2[:, w * BLK:(w + 1) * BLK],
                    in_=dA.rearrange("p (a b) -> p a b", b=D),
                    axis=mybir.AxisListType.X, op=ALU.add)

            # phase 2: pairwise softmax straight from d2 (bias cancels)
            pe = cst.tile([P, sc_cols], F32, name="pe")
            po = cst.tile([P, sc_cols], F32, name="po")
            nc.scalar.activation(pe[:, :], d2[:, :], AF.Sigmoid)
            nc.scalar.activation(po[:, :], d2[:, :], AF.Sigmoid, scale=-1.0)
            me = cst.tile([P, sc_cols], I8, name="me")
            mo = cst.tile([P, sc_cols], I8, name="mo")
            nc.vector.tensor_scalar(me[:, :], d2[:, :], 0.0, None, ALU.is_gt)
            nc.vector.tensor_scalar(mo[:, :], d2[:, :], 0.0, None, ALU.is_lt)

            nc.sync.dma_start(out=pe_out[:, :], in_=pe[:, :])
            nc.sync.dma_start(out=po_out[:, :], in_=po[:, :])
            nc.sync.dma_start(out=me_out[:, :], in_=me[:, :])
            nc.sync.dma_start(out=mo_out[:, :], in_=mo[:, :])

    nc.compile()
    return _split_sync_waits(nc)


def _plan(rows_asaobsbo_by_core):
    """Shared (SPMD) instruction layout from per-core bucket counts.

    rows_asaobsbo_by_core: per core dict with 'key' [npairs] int arrays.
    Returns caps, bucket offsets, s_total, inst list, idx_cols.
    """
    counts = np.zeros((NCORES, 16), np.int64)
    for c in range(NCORES):
        counts[c] = np.bincount(rows_asaobsbo_by_core[c]["key"], minlength=16)
    caps = ((counts.max(axis=0) + P - 1) // P * P).astype(np.int64)
    caps = np.maximum(caps, P)
    s_used = int(caps.sum())
    s_total = (s_used + WSLOTS - 1) // WSLOTS * WSLOTS
    caps[15] += s_total - s_used
    offs = np.zeros(17, np.int64)
    offs[1:] = np.cumsum(caps)

    # stream s: key bit (3-s); window bit of bucket b for stream s:
    def bit(b, s):
        return (b >> (3 - s)) & 1

    insts = []
    idx_cols = 0
    n_win = s_total // WSLOTS
    for w in range(n_win):
        lo_w, hi_w = w * WSLOTS, (w + 1) * WSLOTS
        for s in range(4):
            # runs of consecutive buckets with equal bit
            b = 0
            while b < 16:
                e = b
                while e + 1 < 16 and bit(e + 1, s) == bit(b, s):
                    e += 1
                rlo, rhi = int(offs[b]), int(offs[e + 1])
                a, bnd = max(rlo, lo_w), min(rhi, hi_w)
                if a < bnd:
                    ln = bnd - a
                    insts.append(dict(w=w, s=s, a=(a - lo_w) // P, len=ln,
                                      bit=bit(b, s), col=idx_cols))
                    idx_cols += ln // 16 + 1
                b = e + 1
    return caps, offs, s_total, insts, idx_cols


def kernel(h, edges, W, b):
    h = np.ascontiguousarray(np.asarray(h, dtype=np.float32))
    edges = np.asarray(edges)
    W = np.asarray(W, dtype=np.float32)
    b = np.asarray(b, dtype=np.float32)
    E = edges.shape[1]
    npairs = E // 2
    pp_core = npairs // NCORES
    assert npairs % NCORES == 0

    ev_s = edges[0, 0::2].astype(np.int64)
    ev_o = edges[1, 0::2].astype(np.int64)
    od_s = edges[0, 1::2].astype(np.int64)
    od_o = edges[1, 1::2].astype(np.int64)

    win = lambda r: (r >= WIN_SPLIT).astype(np.int64)
    key_all = (win(ev_s) << 3) | (win(ev_o) << 2) | (win(od_s) << 1) | win(od_o)

    per_core = []
    for c in range(NCORES):
        sl = slice(c * pp_core, (c + 1) * pp_core)
        per_core.append({
            "key": key_all[sl],
            "rows": (ev_s[sl], ev_o[sl], od_s[sl], od_o[sl]),
            "orig": np.arange(c * pp_core, (c + 1) * pp_core, dtype=np.int64),
        })

    caps, offs, s_total, insts, idx_cols = _plan(per_core)
    n_win = s_total // WSLOTS
    sc_cols = s_total // P

    bases = np.array([B0, B1], np.int64)

    # per-core slot assignment + idx blobs
    in_maps = []
    slot_orig = []  # per core: orig pair id per slot (-1 = pad)
    wrep_np = np.tile(W[0], (P, 1)).astype(np.float32)
    wrep_np = np.tile(wrep_np, (1, WSLOTS // D))  # [P, WSLOTS]

    for c in range(NCORES):
        pc = per_core[c]
        order = np.argsort(pc["key"], kind="stable")
        keys_sorted = pc["key"][order]
        # slot for i-th sorted pair: bucket offset + rank within bucket
        kcounts = np.bincount(keys_sorted, minlength=16)
        koffs = np.zeros(16, np.int64)
        koffs[:] = offs[:16]
        rank = np.arange(len(order)) - np.repeat(
            np.cumsum(np.concatenate([[0], kcounts[:-1]])), kcounts)
        slots = koffs[keys_sorted] + rank

        so = np.full(s_total, -1, np.int64)
        so[slots] = pc["orig"][order]
        slot_orig.append(so)

        # per-stream row per slot (pads: row = base row of the bucket's window)
        rows_slot = np.zeros((4, s_total), np.int64)
        for s in range(4):
            rows_slot[s, slots] = pc["rows"][s][order]
        # pads: fill with a row valid for each bucket's window for that stream
        pad_mask = so < 0
        if pad_mask.any():
            bucket_of_slot = np.searchsorted(offs[1:17], np.arange(s_total),
                                             side="right")
            for s in range(4):
                bit_s = (bucket_of_slot >> (3 - s)) & 1
                rows_slot[s, pad_mask] = bases[bit_s[pad_mask]]

        blob = np.zeros((P, idx_cols), np.int16)
        for it in insts:
            s = it["s"]
            lo = it["w"] * WSLOTS + it["a"] * P
            loc = rows_slot[s, lo:lo + it["len"]] - bases[it["bit"]]
            block = np.zeros((16, it["len"] // 16 + 1), np.int16)
            block[:, :-1] = loc.astype(np.int16).reshape(-1, 16).T
            block[0, -1] = 0  # sentinel: keeps trailing index non-negative
            blob[:, it["col"]:it["col"] + block.shape[1]] = np.tile(block, (8, 1))

        in_maps.append({"h": h, "idx": blob, "wrep": wrep_np})

    bias_val = float(b.reshape(-1)[0]) if b.size else 0.0
    nc = _build_program(s_total, insts, idx_cols, n_nodes=h.shape[0],
                        bases=(B0, B1), bias_val=bias_val)
    res = run_bass_kernel_spmd(nc, in_maps, core_ids=list(range(NCORES)))

    score = np.zeros(E, np.float32)
    mask = np.zeros(E, np.uint8)
    for c in range(NCORES):
        out = res.results[c]
        # [P, sc_cols] -> slot order: slot = (col//BLK)*WSLOTS + (col%BLK)*P + p
        def to_slots(arr):
            return arr.reshape(P, n_win, BLK).transpose(1, 2, 0).reshape(-1)
        pe = to_slots(out["pe_out"])
        po = to_slots(out["po_out"])
        me = to_slots(out["me_out"])
        mo = to_slots(out["mo_out"])
        so = slot_orig[c]
        v = so >= 0
        op = so[v]
        score[2 * op] = pe[v]
        score[2 * op + 1] = po[v]
        mask[2 * op] = me[v]
        mask[2 * op + 1] = mo[v]

    return score.reshape(E, 1), mask.reshape(E, 1).astype(bool)


# revision 17
# speedup vs baseline: 1.0461x; 1.0451x over previous
"""Trainium2 Bass kernel for nn_SinePredictor (gnn_message_passing).

reference:
    s = h[edges[0]]; o = h[edges[1]]
    score = sin(s - o) @ W.T + b          # [E, 1]
    score = softmax(score.reshape(-1,2), axis=1).reshape(-1,1)
    return (score, score > 0.5)

Strategy (8 NeuronCores, SPMD):
  - Shard pairs (consecutive edge couples) across 8 cores; replicate h, W.
  - Device-side row gather via gpsimd.dma_gather (int16 indices).  h has
    100000 rows > int16 range, so two 65536-row windows with mid-table
    biased bases (signed int16 reach) cover it:
        W0: base row 32768 -> rows [0, 65536)
        W1: base row 67232 -> rows [34464, 100000)
  - dma_gather requires each instruction's index list to be window-pure, so
    pairs are bucket-sorted host-side by the 4-bit key (window of even-s,
    even-o, odd-s, odd-o).  Even edges live in slot-space A, odd edges in
    slot-space B, aligned by pair slot, so the pairwise softmax is a pure
    elementwise op between the two spaces.  Final outputs are unpermuted on
    the host (pure data movement).
  - Four gather streams (A-s, A-o, B-s, B-o) ride four SWDGE queues
    (descriptor generation runs on different Q7 core pairs in parallel).
  - sin: ACT table is ~1 ulp for |x| <= pi only; args reach +-8.4, so
    range-reduce with k = round(x/(2pi)) via the +-1.5*2^23 magic-add trick,
    r = x - k*f32(2pi).
  - only the pair difference d2 is ever needed: (sin(dA)-sin(dB)) * W,
    reduced over D on DVE (one mul+reduce instead of two score pipelines).
  - pair softmax: softmax([a,b]) = [sigmoid(a-b), sigmoid(b-a)]; bool output
    from the sign of d2 = a-b directly (exactly matches p > 0.5).
"""
import numpy as np

import concourse.bacc as bacc
import concourse.mybir as mybir
import concourse.tile as tile
from concourse.bass_utils import run_bass_kernel_spmd

NCORES = 8
P = 128
D = 128
N_NODES = 100000
B0 = 32768            # window-0 base row (covers rows [0, 65536))
B1 = 67232            # window-1 base row (covers rows [34464, 100000))
WIN_SPLIT = 65536     # rows < WIN_SPLIT use window 0, else window 1
WSLOTS = 896          # pair slots per compute window; len+1 sentinel keeps each
                      # dma_gather at <= 8 x 128-blocks (65 descs/engine, HW-safe)
BLK = WSLOTS // P     # 128-slot blocks per window (8)

MAGIC = 12582912.0            # 1.5 * 2^23: add+sub rounds f32 to nearest int
INV2PI = float(np.float32(1.0 / (2.0 * np.pi)))
C1 = 6.28125                  # exact in f32 (11-bit mantissa)
C2 = float(np.float32(2.0 * np.pi - 6.28125))
C1F = float(np.float32(2.0 * np.pi))   # single-step Cody-Waite constant

# Gray-code layout order for the 16 buckets: adjacent buckets differ in one
# window bit, minimizing per-stream run transitions (26 -> 15) and thus the
# number of split gather instructions.
GRAY = np.array([i ^ (i >> 1) for i in range(16)], np.int64)
LPOS = np.argsort(GRAY)   # LPOS[key] = layout position of bucket `key`

F32 = mybir.dt.float32
I16 = mybir.dt.int16
I8 = mybir.dt.int8
AF = mybir.ActivationFunctionType
ALU = mybir.AluOpType


def _split_sync_waits(nc, max_waits=1):
    """Walrus limits sync-wait slots per instruction (1 for DMA structs).
    Move excess waits onto preceding same-engine NoOps."""
    for func in nc.m.functions:
        for blk in func.blocks:
            insts = blk.instructions
            i = 0
            while i < len(insts):
                inst = insts[i]
                si = inst.sync_info
                if si is not None and len(si.on_wait) > max_waits:
                    waits = list(si.on_wait)
                    keep = waits[-max_waits:]
                    excess = waits[:-max_waits]
                    n_nops = 0
                    for cs in range(0, len(excess), max_waits):
                        chunk = excess[cs:cs + max_waits]
                        nop = mybir.InstNoOp(
                            name=nc.get_next_instruction_name(), ins=[], outs=[])
                        nop.engine = inst.engine
                        nop.sync_info = mybir.SyncInfo(on_wait=chunk, on_update=[])
                        nc.register_instruction(nop)
                        insts.insert(i + n_nops, nop)
                        n_nops += 1
                    si.on_wait = keep
                    i += n_nops
                i += 1
    return nc


def _build_program(s_total, insts, idx_cols, n_nodes=N_NODES, bases=(B0, B1), bias_val=0.0):
    """Build the SPMD Bass program.

    insts: list of dicts(w, s, a, len, bit, col) — identical across cores.
    """
    n_win = s_total // WSLOTS
    sc_cols = s_total // P

    nc = bacc.Bacc(None, target_bir_lowering=False, num_swdge_queues=4)

    h = nc.dram_tensor("h", [n_nodes, D], F32, kind="ExternalInput")
    idx = nc.dram_tensor("idx", [P, idx_cols], I16, kind="ExternalInput")
    wrep = nc.dram_tensor("wrep", [P, WSLOTS], F32, kind="ExternalInput")

    pe_out = nc.dram_tensor("pe_out", [P, sc_cols], F32, kind="ExternalOutput")
    po_out = nc.dram_tensor("po_out", [P, sc_cols], F32, kind="ExternalOutput")
    me_out = nc.dram_tensor("me_out", [P, sc_cols], I8, kind="ExternalOutput")
    mo_out = nc.dram_tensor("mo_out", [P, sc_cols], I8, kind="ExternalOutput")

    h_base = [h[bases[0]:, :], h[bases[1]:, :]]

    by_win = [[] for _ in range(n_win)]
    for it in insts:
        by_win[it["w"]].append(it)

    with tile.TileContext(nc) as tc:
        with tc.tile_pool(name="cst", bufs=1) as cst, \
             tc.tile_pool(name="gpool", bufs=7) as gpool, \
             tc.tile_pool(name="pool", bufs=3) as pool:
            idx_t = cst.tile([P, idx_cols], I16, name="idx_t")
            nc.sync.dma_start(out=idx_t[:, :], in_=idx[:, :])
            wrep_t = cst.tile([P, WSLOTS], F32, name="wrep_t")
            nc.sync.dma_start(out=wrep_t[:, :], in_=wrep[:, :])

            d2 = cst.tile([P, sc_cols], F32, name="d2")

            for w in range(n_win):
                # gather tiles: one spare block for the sentinel slot
                g = {}
                for sname, snum in (("As", 0), ("Ao", 1), ("Bs", 2), ("Bo", 3)):
                    g[snum] = gpool.tile([P, BLK + 1, D], F32,
                                         name=f"g{sname}", tag=f"g{sname}")
                for it in by_win[w]:
                    nblk = it["len"] // P
                    nc.gpsimd.dma_gather(
                        out_ap=g[it["s"]][:, it["a"]:it["a"] + nblk + 1, :],
                        in_ap=h_base[it["bit"]],
                        idxs_ap=idx_t[:, it["col"]:it["col"] + it["len"] // 16 + 1],
                        num_idxs=it["len"] + 1,
                        num_idxs_reg=it["len"] + 1,
                        elem_size=D,
                        queue_num=it["s"],
                    )

                sins = []
                for sp, (gs, go) in enumerate(((0, 1), (2, 3))):
                    dt = pool.tile([P, WSLOTS], F32, name=f"d{sp}", tag=f"d{sp}")
                    tt = pool.tile([P, WSLOTS], F32, name=f"t{sp}", tag=f"t{sp}")
                    gsv = g[gs].rearrange("p a b -> p (a b)")
                    gov = g[go].rearrange("p a b -> p (a b)")
                    # d = s - o
                    nc.vector.tensor_sub(dt[:, :], gsv[:, :WSLOTS], gov[:, :WSLOTS])
                    # t = d*inv2pi + MAGIC ; k = t - MAGIC (both on ACT)
                    nc.scalar.activation(tt[:, :], dt[:, :], AF.Copy,
                                         bias=MAGIC, scale=INV2PI)
                    nc.scalar.activation(tt[:, :], tt[:, :], AF.Copy,
                                         bias=-MAGIC, scale=1.0)
                    # r = d - k*(C1+C2)  (single-step Cody-Waite, in place)
                    nc.vector.scalar_tensor_tensor(
                        out=dt[:, :], in0=tt[:, :], scalar=-C1F, in1=dt[:, :],
                        op0=ALU.mult, op1=ALU.add)
                    # sin
                    nc.scalar.activation(tt[:, :], dt[:, :], AF.Sin)
                    sins.append((dt, tt))
                # d2 terms: (sinA - sinB) * W, reduced over D
                dA, sA = sins[0]
                dB, sB = sins[1]
                nc.vector.tensor_sub(dA[:, :], sA[:, :], sB[:, :])
                nc.vector.tensor_mul(dA[:, :], dA[:, :], wrep_t[:, :])
                nc.vector.tensor_reduce(
                    out=d2[:, w * BLK:(w + 1) * BLK],
                    in_=dA.rearrange("p (a b) -> p a b", b=D),
                    axis=mybir.AxisListType.X, op=ALU.add)

            # phase 2: pairwise softmax straight from d2 (bias cancels)
            pe = cst.tile([P, sc_cols], F32, name="pe")
            po = cst.tile([P, sc_cols], F32, name="po")
            nc.scalar.activation(pe[:, :], d2[:, :], AF.Sigmoid)
            nc.scalar.activation(po[:, :], d2[:, :], AF.Sigmoid, scale=-1.0)
            me = cst.tile([P, sc_cols], I8, name="me")
            mo = cst.tile([P, sc_cols], I8, name="mo")
            nc.vector.tensor_scalar(me[:, :], d2[:, :], 0.0, None, ALU.is_gt)
            nc.vector.tensor_scalar(mo[:, :], d2[:, :], 0.0, None, ALU.is_lt)

            nc.sync.dma_start(out=pe_out[:, :], in_=pe[:, :])
            nc.sync.dma_start(out=po_out[:, :], in_=po[:, :])
            nc.sync.dma_start(out=me_out[:, :], in_=me[:, :])
            nc.sync.dma_start(out=mo_out[:, :], in_=mo[:, :])

    nc.compile()
    return _split_sync_waits(nc)


def _plan(rows_asaobsbo_by_core):
    """Shared (SPMD) instruction layout from per-core bucket counts.

    rows_asaobsbo_by_core: per core dict with 'key' [npairs] int arrays.
    Returns caps, bucket offsets, s_total, inst list, idx_cols.
    """
    counts = np.zeros((NCORES, 16), np.int64)
    for c in range(NCORES):
        counts[c] = np.bincount(LPOS[rows_asaobsbo_by_core[c]["key"]], minlength=16)
    caps = ((counts.max(axis=0) + P - 1) // P * P).astype(np.int64)
    caps = np.maximum(caps, P)
    s_used = int(caps.sum())
    s_total = (s_used + WSLOTS - 1) // WSLOTS * WSLOTS
    caps[15] += s_total - s_used
    offs = np.zeros(17, np.int64)
    offs[1:] = np.cumsum(caps)

    # window bit of layout-position j for stream s (via its Gray-coded key):
    def bit(j, s):
        return int((GRAY[j] >> (3 - s)) & 1)

    insts = []
    idx_cols = 0
    n_win = s_total // WSLOTS
    for w in range(n_win):
        lo_w, hi_w = w * WSLOTS, (w + 1) * WSLOTS
        for s in range(4):
            # runs of consecutive buckets with equal bit
            b = 0
            while b < 16:
                e = b
                while e + 1 < 16 and bit(e + 1, s) == bit(b, s):
                    e += 1
                rlo, rhi = int(offs[b]), int(offs[e + 1])
                a, bnd = max(rlo, lo_w), min(rhi, hi_w)
                if a < bnd:
                    ln = bnd - a
                    insts.append(dict(w=w, s=s, a=(a - lo_w) // P, len=ln,
                                      bit=bit(b, s), col=idx_cols))
                    idx_cols += ln // 16 + 1
                b = e + 1
    return caps, offs, s_total, insts, idx_cols


def kernel(h, edges, W, b):
    h = np.ascontiguousarray(np.asarray(h, dtype=np.float32))
    edges = np.asarray(edges)
    W = np.asarray(W, dtype=np.float32)
    b = np.asarray(b, dtype=np.float32)
    E = edges.shape[1]
    npairs = E // 2
    pp_core = npairs // NCORES
    assert npairs % NCORES == 0

    ev_s = edges[0, 0::2].astype(np.int64)
    ev_o = edges[1, 0::2].astype(np.int64)
    od_s = edges[0, 1::2].astype(np.int64)
    od_o = edges[1, 1::2].astype(np.int64)

    win = lambda r: (r >= WIN_SPLIT).astype(np.int64)
    key_all = (win(ev_s) << 3) | (win(ev_o) << 2) | (win(od_s) << 1) | win(od_o)

    per_core = []
    for c in range(NCORES):
        sl = slice(c * pp_core, (c + 1) * pp_core)
        per_core.append({
            "key": key_all[sl],
            "rows": (ev_s[sl], ev_o[sl], od_s[sl], od_o[sl]),
            "orig": np.arange(c * pp_core, (c + 1) * pp_core, dtype=np.int64),
        })

    caps, offs, s_total, insts, idx_cols = _plan(per_core)
    n_win = s_total // WSLOTS
    sc_cols = s_total // P

    bases = np.array([B0, B1], np.int64)

    # per-core slot assignment + idx blobs
    in_maps = []
    slot_orig = []  # per core: orig pair id per slot (-1 = pad)
    wrep_np = np.tile(W[0], (P, 1)).astype(np.float32)
    wrep_np = np.tile(wrep_np, (1, WSLOTS // D))  # [P, WSLOTS]

    for c in range(NCORES):
        pc = per_core[c]
        mkey = LPOS[pc["key"]]
        order = np.argsort(mkey, kind="stable")
        keys_sorted = mkey[order]
        # slot for i-th sorted pair: bucket offset + rank within bucket
        kcounts = np.bincount(keys_sorted, minlength=16)
        koffs = np.zeros(16, np.int64)
        koffs[:] = offs[:16]
        rank = np.arange(len(order)) - np.repeat(
            np.cumsum(np.concatenate([[0], kcounts[:-1]])), kcounts)
        slots = koffs[keys_sorted] + rank

        so = np.full(s_total, -1, np.int64)
        so[slots] = pc["orig"][order]
        slot_orig.append(so)

        # per-stream row per slot (pads: row = base row of the bucket's window)
        rows_slot = np.zeros((4, s_total), np.int64)
        for s in range(4):
            rows_slot[s, slots] = pc["rows"][s][order]
        # pads: fill with a row valid for each bucket's window for that stream
        pad_mask = so < 0
        if pad_mask.any():
            bucket_of_slot = np.searchsorted(offs[1:17], np.arange(s_total),
                                             side="right")
            for s in range(4):
                bit_s = (GRAY[bucket_of_slot] >> (3 - s)) & 1
                rows_slot[s, pad_mask] = bases[bit_s[pad_mask]]

        blob = np.zeros((P, idx_cols), np.int16)
        for it in insts:
            s = it["s"]
            lo = it["w"] * WSLOTS + it["a"] * P
            loc = rows_slot[s, lo:lo + it["len"]] - bases[it["bit"]]
            block = np.zeros((16, it["len"] // 16 + 1), np.int16)
            block[:, :-1] = loc.astype(np.int16).reshape(-1, 16).T
            block[0, -1] = 0  # sentinel: keeps trailing index non-negative
            blob[:, it["col"]:it["col"] + block.shape[1]] = np.tile(block, (8, 1))

        in_maps.append({"h": h, "idx": blob, "wrep": wrep_np})

    bias_val = float(b.reshape(-1)[0]) if b.size else 0.0
    nc = _build_program(s_total, insts, idx_cols, n_nodes=h.shape[0],
                        bases=(B0, B1), bias_val=bias_val)
    res = run_bass_kernel_spmd(nc, in_maps, core_ids=list(range(NCORES)))

    score = np.zeros(E, np.float32)
    mask = np.zeros(E, np.uint8)
    for c in range(NCORES):
        out = res.results[c]
        # [P, sc_cols] -> slot order: slot = (col//BLK)*WSLOTS + (col%BLK)*P + p
        def to_slots(arr):
            return arr.reshape(P, n_win, BLK).transpose(1, 2, 0).reshape(-1)
        pe = to_slots(out["pe_out"])
        po = to_slots(out["po_out"])
        me = to_slots(out["me_out"])
        mo = to_slots(out["mo_out"])
        so = slot_orig[c]
        v = so >= 0
        op = so[v]
        score[2 * op] = pe[v]
        score[2 * op + 1] = po[v]
        mask[2 * op] = me[v]
        mask[2 * op + 1] = mo[v]

    return score.reshape(E, 1), mask.reshape(E, 1).astype(bool)
